# revision 26
# baseline (speedup 1.0000x reference)
"""Masked multi-head attention (sparse_attention) on 8 trn2 NeuronCores.

Sharding: query rows are split 8 ways (512 rows per core); every core
computes all 8 heads for its rows, so each core reads only its
[512, 4096] slice of the adjacency mask.

Device algorithm per core (scores kept transposed, [keys, queries]):
  qT = Wq^T @ xT[:, my_cols] + bq     [256, 512]  fp16 (fp32 accum)
  kT = Wk^T @ xT + bk                 [256, 4096] fp16
  V' = [x @ Wv | ones | junk] per head  [4096, 8*64] fp16
       (bv is folded into bo on the host: (O/r+bv)Wo+bo = (O/r)Wo+bo')
  for head group G (heads 4G..4G+3), k-tile t (128 keys):
      S^T = K_h Q_h^T  (4 heads row-tiled on PE, K=32, emitted
            back-to-back for row-group concurrency) -> 4 PSUM banks
      per-key-tile path (masking+exp split across ACT and DVE):
        alpha: E = exp(S^T/sqrt(32)) on ACT; P = E*adjT on DVE (fp16 2x)
        beta : P_bits = int16(S^T*c1 + Mbeta) on DVE (one op);
               bitcast fp16 == 2^(S*scale*log2e) Schraudolph approx,
               masked entries saturate to -32768 = fp16 -0.0
      bank += [O_h; r_h] = V'^T @ P  (col-tiled M=64, ones col gives r)
  per bank: broadcast r via Sel matmul, reciprocal, scale O strips;
  final^T = Wo4^T @ oTb + bo; DMA out transposed [256, 512] (host .T).
"""

import contextlib
import sys

import numpy as np

for _p in ("/opt/trn_rl_repo",):
    if _p not in sys.path:
        sys.path.insert(0, _p)

import concourse.bass as bass
import concourse.mybir as mybir
import concourse.tile as tile
from concourse import bacc, bass_utils

F16 = mybir.dt.float16
F32 = mybir.dt.float32
F32R = mybir.dt.float32r
I16 = mybir.dt.int16
AF = mybir.ActivationFunctionType
ALU = mybir.AluOpType

N = 4096
D = 256
H = 8
HD = 32
NCORES = 8
NQ = N // NCORES  # 512 queries per core
KT = N // 128  # 32 key tiles
SCALE = float(1.0 / np.sqrt(np.float32(HD)))
C1 = float(1024.0 * np.log2(np.e) * SCALE)  # schraudolph scale
C2 = 15312.0  # schraudolph bias (fp16-exact)
MASKNEG = -60000.0  # beta-path masked additive (saturates to -0.0)
BIGU = 200.0  # gamma-path additive mask magnitude (unscaled domain)
W64 = 64  # per-head stationary block: [V(32) | ones | junk(31)]

# per-key-tile elementwise path: 0=alpha(ACT+DVE) 1=beta(DVE) 2=gamma(PE+ACT)
# (24 alpha / 8 beta measured fastest on HW; beta is the Schraudolph
# approximation so its fraction also bounds the output error)
PATHS = [0, 0, 1, 0] * 8
# fold bv into bo on the host (out = (O/r)Wo + (bv@Wo + bo)): drops the
# 16 V-projection bias matmuls from the PE stream
FOLD_BV = True
# store the per-key-tile mask duplicated for both head-pairs instead of
# using a stride-0 broadcast AP as the DVE second operand
DUP_MASK = False
# PV software-pipeline lag (in (P, t) units) and elementwise pool depth
# (3/4 measured fastest: decouples PE's PV from the ACT/DVE chain)
PV_LAG = 3
PPOOL_BUFS = 4
# zero the small oTb junk rows instead of the big V' junk region
# (saves ~11us of GpSimd SBUF writes that contend with DVE's port)
ZERO_OTB = False

DRAM_SPECS = [
    ("xT", [D, N], F16, "ExternalInput"),
    ("xq", [D, NQ], F16, "ExternalInput"),
    ("Wq", [D, D], F16, "ExternalInput"),
    ("Wk", [D, D], F16, "ExternalInput"),
    ("Wv", [D, D], F16, "ExternalInput"),
    ("Wo4", [512, D], F32R, "ExternalInput"),
    ("sel", [128, 128], F32, "ExternalInput"),
    ("negid", [128, 128], F16, "ExternalInput"),
    ("bq2", [128, 2], F32, "ExternalInput"),
    ("bk2", [128, 2], F32, "ExternalInput"),
    ("bo2", [128, 2], F32, "ExternalInput"),
    ("bvb2", [1, 2 * D], F16, "ExternalInput"),
    ("out", [D, NQ], F32, "ExternalOutput"),
]


def declare_drams(nc: bass.Bass):
    d = {name: nc.dram_tensor(name, shape, dt, kind=kind).ap()
         for (name, shape, dt, kind) in DRAM_SPECS}
    mw = 2 * NQ if DUP_MASK else NQ
    d["maskT"] = nc.dram_tensor("maskT", [128, KT * mw], F16,
                                kind="ExternalInput").ap()
    return d


def build_kernel(nc: bass.Bass):
    d = declare_drams(nc)
    with tile.TileContext(nc, num_cores=NCORES) as tc:
        with contextlib.ExitStack() as ctx:
            build_body(ctx, tc, d)
    return nc


def build_body(ctx, tc, d):
    nc = tc.nc
    const = ctx.enter_context(tc.tile_pool(name="const", bufs=1))
    big = ctx.enter_context(tc.tile_pool(name="big", bufs=1))
    ppool = ctx.enter_context(tc.tile_pool(name="p", bufs=PPOOL_BUFS))

    # ---- weights / constants ----
    Wq_s = const.tile([128, 2, D], F16, tag="wq")
    Wk_s = const.tile([128, 2, D], F16, tag="wk")
    Wv_s = const.tile([128, 2, D], F16, tag="wv")
    Wo4_s = const.tile([128, 4, D], F32R, tag="wo4")
    for c in range(2):
        nc.sync.dma_start(Wq_s[:, c], d["Wq"][c * 128:(c + 1) * 128, :])
        nc.sync.dma_start(Wk_s[:, c], d["Wk"][c * 128:(c + 1) * 128, :])
        nc.sync.dma_start(Wv_s[:, c], d["Wv"][c * 128:(c + 1) * 128, :])
    for b in range(4):
        nc.sync.dma_start(Wo4_s[:, b], d["Wo4"][b * 128:(b + 1) * 128, :])
    sel_s = const.tile([128, 128], F32, tag="sel")
    nc.sync.dma_start(sel_s[:], d["sel"])
    negid_s = None
    if any(p == 2 for p in PATHS):
        negid_s = const.tile([128, 128], F16, tag="negid")
        nc.sync.dma_start(negid_s[:], d["negid"])
    bq2_s = const.tile([128, 2], F32, tag="bq2")
    bk2_s = const.tile([128, 2], F32, tag="bk2")
    bo2_s = const.tile([128, 2], F32, tag="bo2")
    nc.sync.dma_start(bq2_s[:], d["bq2"])
    nc.sync.dma_start(bk2_s[:], d["bk2"])
    nc.sync.dma_start(bo2_s[:], d["bo2"])
    bvb2_s = None
    onesrow_s = None
    if not FOLD_BV:
        bvb2_s = const.tile([1, 2 * D], F16, tag="bvb2")
        onesrow_s = const.tile([1, 128], F16, tag="onesrow")
        nc.sync.dma_start(bvb2_s[:], d["bvb2"])
        nc.gpsimd.memset(onesrow_s[:], 1.0)

    # ---- big persistent SBUF tensors ----
    xT_s = big.tile([128, 2, N], F16, tag="xT")
    xq_s = big.tile([128, 2, NQ], F16, tag="xq")
    kT_s = big.tile([128, 2, N], F16, tag="kT")
    qT_s = big.tile([128, 2, NQ], F16, tag="qT")
    V_s = big.tile([128, KT, H * W64], F16, tag="V")
    mw = 2 * NQ if DUP_MASK else NQ
    maskT_s = big.tile([128, KT * mw], F16, tag="maskT")
    for c in range(2):
        nc.sync.dma_start(xq_s[:, c], d["xq"][c * 128:(c + 1) * 128, :])
        nc.sync.dma_start(xT_s[:, c], d["xT"][c * 128:(c + 1) * 128, :])
    # mask in 4 chunks of 8 key-tiles for early availability
    for ch in range(4):
        sl = slice(ch * 8 * mw, (ch + 1) * 8 * mw)
        nc.sync.dma_start(maskT_s[:, sl], d["maskT"][:, sl])

    v4 = V_s[:].rearrange("p t (h w) -> p t h w", w=W64)
    # ones column for the r (softmax denom) rows
    nc.gpsimd.memset(v4[:, :, :, HD:HD + 1], 1.0)
    if not ZERO_OTB:
        # junk cols 33.. zeroed so no NaNs leak through zero Wo4 rows
        nc.gpsimd.memset(v4[:, :, :, HD + 1:], 0.0)

    # ---- phase 1: projections (own scoped PSUM pool) ----
    with tc.tile_pool(name="psm", bufs=4, space="PSUM") as psum_misc:
        def proj_block(w_s, b2_s, src, dst, m, j, on_act):
            pt = psum_misc.tile([128, 512], F32, tag="m")
            for c in range(2):
                nc.tensor.matmul(
                    pt[:],
                    w_s[:, c, m * 128:(m + 1) * 128],
                    src[:, c, j * 512:(j + 1) * 512],
                    start=(c == 0), stop=(c == 1),
                )
            dstap = dst[:, m, j * 512:(j + 1) * 512]
            if on_act:
                nc.scalar.activation(dstap, pt[:], AF.Identity,
                                     bias=b2_s[:, m:m + 1], scale=1.0)
            else:
                nc.vector.tensor_scalar_add(dstap, pt[:], b2_s[:, m:m + 1])

        def emit_v_proj(tp):  # projects key-tiles 2*tp and 2*tp+1
            pt = psum_misc.tile([128, 512], F32, tag="m")
            if FOLD_BV:
                # bv folded into bo on the host; first matmul's start=True
                # clears the whole bank so later regions accumulate cleanly
                for tt in range(2):
                    t = 2 * tp + tt
                    for c in range(2):
                        nc.tensor.matmul(
                            pt[:, tt * D:(tt + 1) * D],
                            xT_s[:, c, t * 128:(t + 1) * 128],
                            Wv_s[:, c],
                            start=(tt == 0 and c == 0),
                            stop=(tt == 1 and c == 1),
                        )
            else:
                nc.tensor.matmul(
                    pt[:], onesrow_s[0:1, :], bvb2_s[0:1, :],
                    start=True, stop=False,
                )
                for tt in range(2):
                    t = 2 * tp + tt
                    for c in range(2):
                        nc.tensor.matmul(
                            pt[:, tt * D:(tt + 1) * D],
                            xT_s[:, c, t * 128:(t + 1) * 128],
                            Wv_s[:, c],
                            start=False, stop=(tt == 1 and c == 1),
                        )
            nc.scalar.copy(
                v4[:, 2 * tp:2 * tp + 2, :, 0:HD],
                pt[:].rearrange("p (t h w) -> p t h w", t=2, w=HD))

        for m in range(2):
            proj_block(Wq_s, bq2_s, xq_s, qT_s, m, 0, on_act=(m == 0))
        for j in range(8):
            proj_block(Wk_s, bk2_s, xT_s, kT_s, 0, j, on_act=(j % 2 == 0))
        for tp in range(16):
            emit_v_proj(tp)
            if tp % 2 == 0:
                proj_block(Wk_s, bk2_s, xT_s, kT_s, 1, tp // 2,
                           on_act=(tp % 4 == 0))

    # ---- phase 2: attention ----
    psum_pv = ctx.enter_context(tc.tile_pool(name="pspv", bufs=1, space="PSUM"))
    psum_qk = ctx.enter_context(tc.tile_pool(name="psqk", bufs=3, space="PSUM"))

    # oTb bank layout: tile b in {0: G0 bankA, 1: G0 bankB, 2: G1 A, 3: G1 B}
    # per bank: partitions 0:32 = O_h(even), 32 = r_h(even), 33:64 junk,
    #           64:96 = O_h(odd), 96 = r_h(odd), 97:128 junk
    oTb = big.tile([128, 4, NQ], F32R, tag="oTb")
    if ZERO_OTB:
        # V' junk cols are uninitialized; keep their products out of oTb
        nc.gpsimd.memset(oTb[33:64, :, :], 0.0)
        nc.gpsimd.memset(oTb[97:128, :, :], 0.0)

    for G in range(2):
        bankA = psum_pv.tile([128, NQ], F32, tag="opsA")
        bankB = psum_pv.tile([128, NQ], F32, tag="opsB")
        banks = [bankA, bankB]

        def emit_pv(P, t, p):
            for ii in range(2):
                h = 4 * G + 2 * P + ii
                nc.tensor.matmul(
                    banks[P][64 * ii:64 * (ii + 1), :],
                    V_s[:, t, W64 * h:W64 * (h + 1)],
                    p[:, ii * NQ:(ii + 1) * NQ],
                    start=(t == 0), stop=(t == KT - 1),
                    tile_position=(0, 64 * ii),
                    skip_group_check=True,
                )

        pending = []  # one-tile software pipeline lag for PV on PE
        for t in range(KT):
            path = PATHS[t]
            mask_t = maskT_s[:, t * mw:t * mw + NQ]
            if DUP_MASK:
                mask_bc = maskT_s[:, t * mw:(t + 1) * mw].rearrange(
                    "q (r c) -> q r c", r=2)
            else:
                mask_bc = mask_t[:, None, :].broadcast_to((128, 2, NQ))
            # all four QK matmuls back-to-back: distinct row groups and
            # distinct PSUM banks -> PE sub-array concurrency
            qks = []
            for P in range(2):
                qk = psum_qk.tile([128, 2 * NQ], F32, tag="qk")
                for ii in range(2):
                    i = 2 * P + ii
                    nc.tensor.matmul(
                        qk[:, ii * NQ:(ii + 1) * NQ],
                        kT_s[32 * i:32 * (i + 1), G, t * 128:(t + 1) * 128],
                        qT_s[32 * i:32 * (i + 1), G, :],
                        start=True, stop=(path != 2),
                        tile_position=(32 * i, 0),
                    )
                qks.append(qk)
            if path == 2:  # gamma: additive mask via PE
                for P in range(2):
                    for ii in range(2):
                        nc.tensor.matmul(
                            qks[P][:, ii * NQ:(ii + 1) * NQ],
                            negid_s[:], mask_t,
                            start=False, stop=True,
                        )
            for P in range(2):
                qk = qks[P]
                if path == 0:  # alpha
                    e = ppool.tile([128, 2 * NQ], F16, tag="e")
                    nc.scalar.activation(e[:], qk[:], AF.Exp, bias=0.0,
                                         scale=SCALE)
                    p = ppool.tile([128, 2 * NQ], F16, tag="p")
                    nc.vector.tensor_tensor(
                        p[:].rearrange("q (r c) -> q r c", r=2),
                        e[:].rearrange("q (r c) -> q r c", r=2),
                        mask_bc, op=ALU.mult)
                    pap = p[:]
                elif path == 1:  # beta
                    pb = ppool.tile([128, 2 * NQ], I16, tag="pb")
                    nc.vector.scalar_tensor_tensor(
                        pb[:].rearrange("q (r c) -> q r c", r=2),
                        qk[:].rearrange("q (r c) -> q r c", r=2),
                        C1, mask_bc, op0=ALU.mult, op1=ALU.add)
                    pap = pb[:].bitcast(F16)
                else:  # gamma
                    p = ppool.tile([128, 2 * NQ], F16, tag="p")
                    nc.scalar.activation(p[:], qk[:], AF.Exp, bias=0.0,
                                         scale=SCALE)
                    pap = p[:]
                pending.append((P, t, pap))
                if len(pending) > PV_LAG:
                    emit_pv(*pending.pop(0))
        for args in pending:
            emit_pv(*args)
        for bk in range(2):
            b = 2 * G + bk
            # broadcast raw r rows (partitions 32 and 96) across halves
            rx = psum_qk.tile([128, 2 * NQ], F32, tag="qk")
            rsb = ppool.tile([128, NQ], F32, tag="rsb")
            nc.gpsimd.memset(rsb[:], 1.0)
            nc.vector.tensor_copy(rsb[32:33, :], banks[bk][32:33, :])
            nc.vector.tensor_copy(rsb[96:97, :], banks[bk][96:97, :])
            nc.tensor.matmul(rx[:, :NQ], sel_s[:], rsb[:], start=True,
                             stop=True)
            rr = ppool.tile([128, NQ], F32, tag="rr")
            nc.vector.reciprocal(rr[:], rx[:, :NQ])
            if ZERO_OTB:
                # restrict to live rows: junk PSUM rows hold garbage
                nc.vector.tensor_tensor(
                    oTb[0:33, b], banks[bk][0:33, :], rr[0:33, :],
                    op=ALU.mult)
                nc.vector.tensor_tensor(
                    oTb[64:97, b], banks[bk][64:97, :], rr[64:97, :],
                    op=ALU.mult)
            else:
                nc.vector.tensor_tensor(
                    oTb[:, b], banks[bk][:], rr[:], op=ALU.mult)

    # ---- output projection (emitted transposed; host untransposes) ----
    fT = big.tile([128, 2, NQ], F32, tag="fT")
    for m in range(2):
        pt = psum_qk.tile([128, 2 * NQ], F32, tag="qk")
        for b in range(4):
            nc.tensor.matmul(pt[:, :NQ], Wo4_s[:, b, m * 128:(m + 1) * 128],
                             oTb[:, b], start=(b == 0), stop=(b == 3))
        nc.vector.tensor_scalar_add(fT[:, m], pt[:, :NQ], bo2_s[:, m:m + 1])
        nc.sync.dma_start(d["out"][m * 128:(m + 1) * 128, :], fT[:, m])


def prepare_in_maps(x, adj, Wq, bq, Wk, bk, Wv, bv, Wo, bo):
    x = np.asarray(x, np.float32)
    adj = np.asarray(adj, np.float32)

    xT = np.ascontiguousarray(x[0].T).astype(np.float16)  # [256, 4096]

    def b2(v):  # [256] -> [128, 2] (per-partition scalar per half)
        return np.ascontiguousarray(
            np.asarray(v, np.float32).reshape(2, 128).T)

    Wo = np.asarray(Wo, np.float32)
    Wo4 = np.zeros((4, 128, D), np.float32)
    for b in range(4):
        G, isB = divmod(b, 2)
        for hh in range(2):
            h = 4 * G + 2 * isB + hh
            Wo4[b, 64 * hh:64 * hh + 32, :] = Wo[32 * h:32 * h + 32, :]
    Wo4 = np.ascontiguousarray(Wo4.reshape(512, D))

    sel = np.zeros((128, 128), np.float32)
    sel[32, 0:64] = 1.0
    sel[96, 64:128] = 1.0

    negid = (np.eye(128) * np.float32(-BIGU)).astype(np.float16)

    bvb2 = np.tile(np.asarray(bv, np.float32).reshape(1, D),
                   (1, 2)).astype(np.float16)
    bo_eff = np.asarray(bo, np.float32)
    if FOLD_BV:
        bo_eff = bo_eff + np.asarray(bv, np.float32) @ Wo

    common = dict(
        xT=xT,
        Wq=np.asarray(Wq, np.float32).astype(np.float16),
        Wk=np.asarray(Wk, np.float32).astype(np.float16),
        Wv=np.asarray(Wv, np.float32).astype(np.float16),
        Wo4=Wo4, sel=sel, negid=negid,
        bq2=b2(bq), bk2=b2(bk), bo2=b2(bo_eff), bvb2=bvb2,
    )
    adjf = adj[0]  # [N, N] rows=queries, cols=keys
    in_maps = []
    for c in range(NCORES):
        m = dict(common)
        m["xq"] = np.ascontiguousarray(xT[:, c * NQ:(c + 1) * NQ])
        # [keys, queries] for this core, tiled [t(32), k(128), q(512)]
        At = adjf[c * NQ:(c + 1) * NQ, :].T.reshape(KT, 128, NQ)
        enc = np.empty((KT, 128, NQ), np.float16)
        for t in range(KT):
            p = PATHS[t]
            if p == 0:
                enc[t] = At[t].astype(np.float16)
            elif p == 1:
                enc[t] = np.where(At[t] > 0, np.float16(C2),
                                  np.float16(MASKNEG))
            else:
                enc[t] = (1.0 - At[t]).astype(np.float16)
        if DUP_MASK:
            enc = np.concatenate([enc, enc], axis=2)  # [KT, 128, 2*NQ]
        m["maskT"] = np.ascontiguousarray(
            enc.transpose(1, 0, 2).reshape(128, -1))
        in_maps.append(m)
    return in_maps


_CACHED = {}


def _get_built():
    if "nc" not in _CACHED:
        nc = bacc.Bacc("TRN2", target_bir_lowering=False, debug=False,
                       num_devices=NCORES)
        build_kernel(nc)
        nc.finalize()
        _CACHED["nc"] = nc
    return _CACHED["nc"]


def kernel(x, adj, Wq, bq, Wk, bk, Wv, bv, Wo, bo, trace=False):
    nc = _get_built()
    in_maps = prepare_in_maps(x, adj, Wq, bq, Wk, bk, Wv, bv, Wo, bo)

    res = bass_utils.run_bass_kernel_spmd(
        nc, in_maps, core_ids=list(range(NCORES)), trace=trace)
    out = np.concatenate(
        [r["out"].T for r in res.results], axis=0)  # [N, D]
    kernel.last_results = res
    return out[None, :, :].astype(np.float32)
